# revision 29
# baseline (speedup 1.0000x reference)
"""Single-head attention (B=8, S=2048, DIN=DK=DV=1024) on 8 TRN2 NeuronCores.

Strategy: pure data-parallel — one batch element per core, identical SPMD
program, no collectives. All layout transposes are done host-side so the
device kernel is transpose-free.

Algebraic restructure (softmax is invariant to per-query constants):
    scores = (xq Wq^T + bq)(xk Wk^T + bk)^T / sqrt(dk)
           = xq M xk^T + e_s + (q-only terms that softmax cancels)
  with M = (Wq^T Wk) / sqrt(dk) folded on the host (weight-only precompute)
  and e_s[s] = xk[s] . (Wk^T bq) / sqrt(dk) computed on host per batch.
  This deletes the entire K projection from the device.

  per core (feature-major layouts, features on partitions):
    GT[d,q]    = M^T.T @ xqT       (the only q-side projection)
    V[s,v]     = xvT.T @ WvT       (bias bv folded into the epilogue)
    ST[s,q]    = xkT.T @ GT        (scores, transposed layout)
    E[s,q]     = exp(ST + e_s)     (e_s fused as per-partition ACT bias;
                                    no max-subtraction: scores are O(1))
    U[q,v]     = E.T @ V           (unnormalized output)
    r[q]       = E.T @ ones        (softmax denominators)
    out[q,v]   = U * (1/r) + bv    (one fused DVE pass per tile)

Matmuls run in bf16 (inputs pre-cast on host), fp32 accumulation in PSUM.
"""

import numpy as np
import ml_dtypes

import concourse.bass as bass
import concourse.tile as tile
from concourse import bacc, mybir
from concourse.bass_utils import run_bass_kernel_spmd

B, S, D = 8, 2048, 1024
N_CORES = 8
PB = 128           # partition block
NCH = 512          # matmul moving-dim / PSUM bank chunk
SB = S // PB       # 16 s-blocks
DB = D // PB       # 8 feature blocks
SCH = S // NCH     # 4 s-chunks
QCH = S // NCH     # 4 q-chunks
VCH = D // NCH     # 2 v-chunks
XSPLIT = ((0, 3), (3, 6), (6, 8))  # x-chunk DMA split across 3 engines

BF16 = mybir.dt.bfloat16
F32 = mybir.dt.float32

_compiled = [None]


def _build():
    nc = bacc.Bacc("TRN2", target_bir_lowering=False, debug=False,
                   num_devices=N_CORES, num_swdge_queues=2,
                   enable_partition_id=False)

    # x*T chunks: [s_chunk][128 part(din)][din_blk][512 s]; per-partition row
    # of a chunk is contiguous (8KB) for DMA efficiency.
    xqT = nc.dram_tensor("xqT", [SCH, PB, DB, NCH], BF16, kind="ExternalInput")
    xkT = nc.dram_tensor("xkT", [SCH, PB, DB, NCH], BF16, kind="ExternalInput")
    xvT = nc.dram_tensor("xvT", [SCH, PB, DB, NCH], BF16, kind="ExternalInput")
    # weight-like matrices, column-blocked: [out_blk][128 part(din)][din_blk][128 out]
    mT = nc.dram_tensor("mT", [DB, PB, DB, PB], BF16, kind="ExternalInput")
    wvT = nc.dram_tensor("wvT", [DB, PB, DB, PB], BF16, kind="ExternalInput")
    esT = nc.dram_tensor("esT", [PB, SB], F32, kind="ExternalInput")  # [p, s_blk]
    bv = nc.dram_tensor("bv", [1, D], F32, kind="ExternalInput")
    out = nc.dram_tensor("out", [S, D], F32, kind="ExternalOutput")

    with tile.TileContext(nc) as tc:
        with (
            tc.tile_pool(name="res", bufs=1) as res,      # phase-resident tensors
            tc.tile_pool(name="wpool", bufs=2) as wpool,  # streamed weights
            tc.tile_pool(name="xpool", bufs=3) as xpool,  # streamed x chunks
            tc.tile_pool(name="epool", bufs=2) as epool,  # exp tiles
            tc.tile_pool(name="opool", bufs=2) as opool,  # output staging
            tc.tile_pool(name="misc", bufs=1) as misc,
            tc.tile_pool(name="psA", bufs=2, space="PSUM") as psA,
            tc.tile_pool(name="psB", bufs=2, space="PSUM") as psB,
        ):
            dmae = [nc.sync, nc.gpsimd, nc.scalar]

            def load_x_chunk(xt_ap, xdram_sc, rot):
                for i, (a, b) in enumerate(XSPLIT):
                    dmae[(i + rot) % 3].dma_start(
                        out=xt_ap[:, a:b], in_=xdram_sc[:, a:b]
                    )

            # ---- constants ----
            bvB = misc.tile([PB, D], F32, tag="bvB")
            ess = misc.tile([PB, SB], F32, tag="es")
            ones = misc.tile([PB, 1], BF16, tag="ones")
            nc.vector.memset(ones[:], 1.0)

            # ---- PE warmup: release the HAM clock throttle while the
            # startup DMAs are in flight (PE is otherwise idle ~7us) ----
            warm = misc.tile([PB, 256], BF16, tag="warm")
            nc.vector.memset(warm[:], 0.0)
            for i in range(24):
                pw = psB.tile([1, 256], F32, tag="pr")
                nc.tensor.matmul(out=pw[:], lhsT=ones[:], rhs=warm[:],
                                 start=True, stop=True)

            # ---- resident tensors ----
            GT = res.tile([PB, DB, S], BF16, tag="GT")   # [p(d), d_blk, q]
            XK = res.tile([PB, DB, S], BF16, tag="XK")   # [p(din), din_blk, s]
            V = res.tile([PB, SB, D], BF16, tag="V")     # [p(s), s_blk, v]

            # ---- G projection (G = xq @ M), startup-critical ----
            # Cold-start choreography: the first PSUM group needs M.col0 +
            # all 8 kb-blocks of xq chunk 0 (~1.25MB); later groups need one
            # more 256KB M column each. Order the queues so data lands just
            # ahead of the PE.
            wt = wpool.tile([PB, DB, DB, PB], BF16, tag="w")
            nc.sync.dma_start(out=wt[:, 0], in_=mT[0])  # first column first
            for sc in range(SCH):
                xt = xpool.tile([PB, DB, NCH], BF16, tag="x")
                if sc == 0:
                    nc.scalar.dma_start(out=xt[:, 0:3], in_=xqT[0][:, 0:3])
                    nc.sync.dma_start(out=xt[:, 3:6], in_=xqT[0][:, 3:6])
                    nc.gpsimd.dma_start(out=xt[:, 6:8], in_=xqT[0][:, 6:8])
                    for db in range(1, DB):
                        dmae[(db % 2) * 2].dma_start(out=wt[:, db], in_=mT[db])
                    # startup-noncritical loads, behind the first blocks
                    nc.gpsimd.dma_start(out=ess[:], in_=esT[:])
                    nc.gpsimd.dma_start(out=bvB[:], in_=bv.ap().to_broadcast((PB, D)))
                else:
                    load_x_chunk(xt, xqT[sc], rot=sc)
                for db in range(DB):
                    pt = psA.tile([PB, NCH], F32, tag="pp")
                    for kb in range(DB):
                        nc.tensor.matmul(
                            out=pt[:],
                            lhsT=wt[:, db, kb, :],
                            rhs=xt[:, kb, :],
                            start=(kb == 0),
                            stop=(kb == DB - 1),
                        )
                    nc.vector.tensor_copy(
                        out=GT[:, db, sc * NCH:(sc + 1) * NCH], in_=pt[:]
                    )

            # ---- load xk directly (no K projection) ----
            for sc in range(SCH):
                load_x_chunk(XK[:, :, sc * NCH:(sc + 1) * NCH], xkT[sc], rot=sc)

            # ---- projection V (input is stationary, weight is moving) ----
            wt = wpool.tile([PB, DB, DB, PB], BF16, tag="w")
            for db in range(DB):
                dmae[db % 2].dma_start(out=wt[:, db], in_=wvT[db])
            for sc in range(SCH):
                xt = xpool.tile([PB, DB, NCH], BF16, tag="x")
                load_x_chunk(xt, xvT[sc], rot=sc)
                for sbl in range(NCH // PB):  # s-blocks within this chunk
                    sb = sc * (NCH // PB) + sbl
                    for vc in range(VCH):
                        pt = psA.tile([PB, NCH], F32, tag="pp")
                        for kb in range(DB):
                            nc.tensor.matmul(
                                out=pt[:],
                                lhsT=xt[:, kb, sbl * PB:(sbl + 1) * PB],
                                rhs=wt[:, 4 * vc:4 * (vc + 1), kb, :],
                                start=(kb == 0),
                                stop=(kb == DB - 1),
                            )
                        nc.vector.tensor_copy(
                            out=V[:, sb, vc * NCH:(vc + 1) * NCH], in_=pt[:]
                        )

            # ---- attention, per q-chunk of 512 ----
            for qc in range(QCH):
                q0 = qc * NCH
                # scores^T [s, q-chunk] then exp(. + e_s) -> E
                E = epool.tile([PB, SB, NCH], BF16, tag="E")
                for sb in range(SB):
                    pt = psA.tile([PB, NCH], F32, tag="ps")
                    for kb in range(DB):
                        nc.tensor.matmul(
                            out=pt[:],
                            lhsT=XK[:, kb, sb * PB:(sb + 1) * PB],
                            rhs=GT[:, kb, q0:q0 + NCH],
                            start=(kb == 0),
                            stop=(kb == DB - 1),
                        )
                    nc.scalar.activation(
                        out=E[:, sb, :], in_=pt[:],
                        func=mybir.ActivationFunctionType.Exp,
                        bias=ess[:, sb:sb + 1],
                    )
                # per q-block of 128: denominators r, then U, then epilogue
                for qb in range(NCH // PB):
                    eq = slice(qb * PB, (qb + 1) * PB)
                    pr = psB.tile([PB, 1], F32, tag="pr")
                    for sb in range(SB):
                        nc.tensor.matmul(
                            out=pr[:], lhsT=E[:, sb, eq], rhs=ones[:],
                            start=(sb == 0), stop=(sb == SB - 1),
                        )
                    recip = misc.tile([PB, 1], F32, tag="recip")
                    nc.vector.reciprocal(out=recip[:], in_=pr[:])
                    qrow = q0 + qb * PB
                    for vc in range(VCH):
                        pu = psB.tile([PB, NCH], F32, tag="pu")
                        for sb in range(SB):
                            nc.tensor.matmul(
                                out=pu[:],
                                lhsT=E[:, sb, eq],
                                rhs=V[:, sb, vc * NCH:(vc + 1) * NCH],
                                start=(sb == 0),
                                stop=(sb == SB - 1),
                            )
                        ot = opool.tile([PB, NCH], F32, tag="ot")
                        nc.vector.scalar_tensor_tensor(
                            out=ot[:],
                            in0=pu[:],
                            scalar=recip[:],
                            in1=bvB[:, vc * NCH:(vc + 1) * NCH],
                            op0=mybir.AluOpType.mult,
                            op1=mybir.AluOpType.add,
                        )
                        h = NCH // 2
                        c0 = vc * NCH
                        nc.sync.dma_start(
                            out=out[qrow:qrow + PB, c0:c0 + h], in_=ot[:, 0:h]
                        )
                        nc.scalar.dma_start(
                            out=out[qrow:qrow + PB, c0 + h:c0 + NCH], in_=ot[:, h:NCH]
                        )

    nc.compile()
    return nc


def _prep_host(query, key_, value, Wq_w, Wq_b, Wk_w, Wk_b, Wv_w, Wv_b):
    """Host-side sharding, layout marshalling, and weight-only algebra."""
    bf16 = ml_dtypes.bfloat16
    scale = np.float32(1.0 / np.sqrt(D))

    def prep_x(x):  # [S, D] fp32 -> [SCH, PB, DB, NCH] bf16 (x.T, blocked)
        # xc[sc, p, kb, n] = x[sc*NCH+n, kb*PB+p]
        return np.ascontiguousarray(
            x.reshape(SCH, NCH, DB, PB).transpose(0, 3, 2, 1)
        ).astype(bf16)

    def prep_w(wT):  # [Din, Dout] fp32 -> [DB, PB, DB, PB] bf16 (blocked)
        # wc[db, p, kb, j] = wT[kb*PB+p, db*PB+j]
        return np.ascontiguousarray(
            wT.reshape(DB, PB, DB, PB).transpose(2, 1, 0, 3)
        ).astype(bf16)

    # weight-only precompute: M = (Wq^T Wk) * scale  [din, din]
    M = (Wq_w.T @ Wk_w) * scale
    mc = prep_w(M)
    wv = prep_w(np.ascontiguousarray(Wv_w.T))
    c = (Wk_w.T @ Wq_b) * scale  # [din]; e_s = xk @ c
    bvr = np.ascontiguousarray(Wv_b.reshape(1, D)).astype(np.float32)

    in_maps = []
    for b in range(B):
        e_s = (key_[b] @ c).astype(np.float32)  # [S]
        in_maps.append({
            "xqT": prep_x(query[b]),
            "xkT": prep_x(key_[b]),
            "xvT": prep_x(value[b]),
            "mT": mc, "wvT": wv,
            "esT": np.ascontiguousarray(e_s.reshape(SB, PB).T),
            "bv": bvr,
        })
    return in_maps


def kernel(query, key_, value, Wq_w, Wq_b, Wk_w, Wk_b, Wv_w, Wv_b):
    query = np.asarray(query, np.float32)
    key_ = np.asarray(key_, np.float32)
    value = np.asarray(value, np.float32)
    Wq_w = np.asarray(Wq_w, np.float32)
    Wq_b = np.asarray(Wq_b, np.float32)
    Wk_w = np.asarray(Wk_w, np.float32)
    Wk_b = np.asarray(Wk_b, np.float32)
    Wv_w = np.asarray(Wv_w, np.float32)
    Wv_b = np.asarray(Wv_b, np.float32)

    if _compiled[0] is None:
        _compiled[0] = _build()
    nc = _compiled[0]

    in_maps = _prep_host(query, key_, value, Wq_w, Wq_b, Wk_w, Wk_b, Wv_w, Wv_b)
    last_err = None
    for attempt in range(3):
        try:
            res = run_bass_kernel_spmd(nc, in_maps, list(range(N_CORES)))
            out = np.stack([res.results[i]["out"] for i in range(N_CORES)], axis=0)
            if np.isfinite(out).all():
                return out
            last_err = RuntimeError("non-finite values in device output")
        except Exception as e:  # transient device errors (e.g. NRT exec unit)
            last_err = e
    raise last_err


# revision 30
# speedup vs baseline: 1.0032x; 1.0032x over previous
"""Single-head attention (B=8, S=2048, DIN=DK=DV=1024) on 8 TRN2 NeuronCores.

Strategy: pure data-parallel — one batch element per core, identical SPMD
program, no collectives. All layout transposes are done host-side so the
device kernel is transpose-free.

Algebraic restructure (softmax is invariant to per-query constants):
    scores = (xq Wq^T + bq)(xk Wk^T + bk)^T / sqrt(dk)
           = xq M xk^T + e_s + (q-only terms that softmax cancels)
  with M = (Wq^T Wk) / sqrt(dk) folded on the host (weight-only precompute)
  and e_s[s] = xk[s] . (Wk^T bq) / sqrt(dk) computed on host per batch.
  This deletes the entire K projection from the device.

  per core (feature-major layouts, features on partitions):
    GT[d,q]    = M^T.T @ xqT       (the only q-side projection)
    V[s,v]     = xvT.T @ WvT       (bias bv folded into the epilogue)
    ST[s,q]    = xkT.T @ GT        (scores, transposed layout)
    E[s,q]     = exp(ST + e_s)     (e_s fused as per-partition ACT bias;
                                    no max-subtraction: scores are O(1))
    U[q,v]     = E.T @ V           (unnormalized output)
    r[q]       = E.T @ ones        (softmax denominators)
    out[q,v]   = U * (1/r) + bv    (one fused DVE pass per tile)

Matmuls run in bf16 (inputs pre-cast on host), fp32 accumulation in PSUM.
"""

import numpy as np
import ml_dtypes

import concourse.bass as bass
import concourse.tile as tile
from concourse import bacc, mybir
from concourse.bass_utils import run_bass_kernel_spmd

B, S, D = 8, 2048, 1024
N_CORES = 8
PB = 128           # partition block
NCH = 512          # matmul moving-dim / PSUM bank chunk
SB = S // PB       # 16 s-blocks
DB = D // PB       # 8 feature blocks
SCH = S // NCH     # 4 s-chunks
QCH = S // NCH     # 4 q-chunks
VCH = D // NCH     # 2 v-chunks
XSPLIT = ((0, 3), (3, 6), (6, 8))  # x-chunk DMA split across 3 engines

BF16 = mybir.dt.bfloat16
F32 = mybir.dt.float32

_compiled = [None]


def _build():
    nc = bacc.Bacc("TRN2", target_bir_lowering=False, debug=False,
                   num_devices=N_CORES, num_swdge_queues=2,
                   enable_partition_id=False)

    # x*T chunks: [s_chunk][128 part(din)][din_blk][512 s]; per-partition row
    # of a chunk is contiguous (8KB) for DMA efficiency.
    xqT = nc.dram_tensor("xqT", [SCH, PB, DB, NCH], BF16, kind="ExternalInput")
    xkT = nc.dram_tensor("xkT", [SCH, PB, DB, NCH], BF16, kind="ExternalInput")
    xvT = nc.dram_tensor("xvT", [SCH, PB, DB, NCH], BF16, kind="ExternalInput")
    # weight-like matrices, column-blocked: [out_blk][128 part(din)][din_blk][128 out]
    mT = nc.dram_tensor("mT", [DB, PB, DB, PB], BF16, kind="ExternalInput")
    wvT = nc.dram_tensor("wvT", [DB, PB, DB, PB], BF16, kind="ExternalInput")
    esT = nc.dram_tensor("esT", [PB, SB], F32, kind="ExternalInput")  # [p, s_blk]
    bv = nc.dram_tensor("bv", [1, D], F32, kind="ExternalInput")
    out = nc.dram_tensor("out", [S, D], F32, kind="ExternalOutput")

    with tile.TileContext(nc) as tc:
        with (
            tc.tile_pool(name="res", bufs=1) as res,      # phase-resident tensors
            tc.tile_pool(name="wpool", bufs=2) as wpool,  # streamed weights
            tc.tile_pool(name="xpool", bufs=3) as xpool,  # streamed x chunks
            tc.tile_pool(name="epool", bufs=2) as epool,  # exp tiles
            tc.tile_pool(name="opool", bufs=2) as opool,  # output staging
            tc.tile_pool(name="misc", bufs=1) as misc,
            tc.tile_pool(name="psA", bufs=2, space="PSUM") as psA,
            tc.tile_pool(name="psB", bufs=2, space="PSUM") as psB,
        ):
            dmae = [nc.sync, nc.gpsimd, nc.scalar]

            def load_x_chunk(xt_ap, xdram_sc, rot):
                for i, (a, b) in enumerate(XSPLIT):
                    dmae[(i + rot) % 3].dma_start(
                        out=xt_ap[:, a:b], in_=xdram_sc[:, a:b]
                    )

            # ---- constants ----
            bvB = misc.tile([PB, D], F32, tag="bvB")
            ess = misc.tile([PB, SB], F32, tag="es")
            ones = misc.tile([PB, 1], BF16, tag="ones")
            nc.vector.memset(ones[:], 1.0)

            # ---- PE warmup: release the HAM clock throttle while the
            # startup DMAs are in flight (PE is otherwise idle ~7us) ----
            warm = misc.tile([PB, 256], BF16, tag="warm")
            nc.vector.memset(warm[:], 0.0)
            for i in range(24):
                pw = psB.tile([1, 256], F32, tag="pr")
                nc.tensor.matmul(out=pw[:], lhsT=ones[:], rhs=warm[:],
                                 start=True, stop=True)

            # ---- resident tensors ----
            GT = res.tile([PB, DB, S], BF16, tag="GT")   # [p(d), d_blk, q]
            XK = res.tile([PB, DB, S], BF16, tag="XK")   # [p(din), din_blk, s]
            V = res.tile([PB, SB, D], BF16, tag="V")     # [p(s), s_blk, v]

            # ---- G projection (G = xq @ M), startup-critical ----
            # Cold-start choreography: the first PSUM group needs M.col0 +
            # all 8 kb-blocks of xq chunk 0 (~1.25MB); later groups need one
            # more 256KB M column each. Order the queues so data lands just
            # ahead of the PE.
            wt = wpool.tile([PB, DB, DB, PB], BF16, tag="w")
            nc.sync.dma_start(out=wt[:, 0], in_=mT[0])  # first column first
            for sc in range(SCH):
                xt = xpool.tile([PB, DB, NCH], BF16, tag="x")
                if sc == 0:
                    nc.scalar.dma_start(out=xt[:, 0:3], in_=xqT[0][:, 0:3])
                    nc.sync.dma_start(out=xt[:, 3:6], in_=xqT[0][:, 3:6])
                    nc.gpsimd.dma_start(out=xt[:, 6:8], in_=xqT[0][:, 6:8])
                    for db in range(1, DB):
                        dmae[(db % 2) * 2].dma_start(out=wt[:, db], in_=mT[db])
                    # startup-noncritical loads, behind the first blocks
                    nc.gpsimd.dma_start(out=ess[:], in_=esT[:])
                    nc.gpsimd.dma_start(out=bvB[:], in_=bv.ap().to_broadcast((PB, D)))
                else:
                    load_x_chunk(xt, xqT[sc], rot=sc)
                for db in range(DB):
                    pt = psA.tile([PB, NCH], F32, tag="pp")
                    for kb in range(DB):
                        nc.tensor.matmul(
                            out=pt[:],
                            lhsT=wt[:, db, kb, :],
                            rhs=xt[:, kb, :],
                            start=(kb == 0),
                            stop=(kb == DB - 1),
                        )
                    nc.vector.tensor_copy(
                        out=GT[:, db, sc * NCH:(sc + 1) * NCH], in_=pt[:]
                    )

            # ---- load xk directly (no K projection) ----
            for sc in range(SCH):
                load_x_chunk(XK[:, :, sc * NCH:(sc + 1) * NCH], xkT[sc], rot=sc)

            # ---- projection V (input is stationary, weight is moving) ----
            wt = wpool.tile([PB, DB, DB, PB], BF16, tag="w")
            for db in range(DB):
                dmae[db % 2].dma_start(out=wt[:, db], in_=wvT[db])
            for sc in range(SCH):
                xt = xpool.tile([PB, DB, NCH], BF16, tag="x")
                load_x_chunk(xt, xvT[sc], rot=sc)
                for sbl in range(NCH // PB):  # s-blocks within this chunk
                    sb = sc * (NCH // PB) + sbl
                    for vc in range(VCH):
                        pt = psA.tile([PB, NCH], F32, tag="pp")
                        for kb in range(DB):
                            nc.tensor.matmul(
                                out=pt[:],
                                lhsT=xt[:, kb, sbl * PB:(sbl + 1) * PB],
                                rhs=wt[:, 4 * vc:4 * (vc + 1), kb, :],
                                start=(kb == 0),
                                stop=(kb == DB - 1),
                            )
                        nc.vector.tensor_copy(
                            out=V[:, sb, vc * NCH:(vc + 1) * NCH], in_=pt[:]
                        )

            # ---- attention, per q-chunk of 512 ----
            for qc in range(QCH):
                q0 = qc * NCH
                # scores^T [s, q-chunk] then exp(. + e_s) -> E
                E = epool.tile([PB, SB, NCH], BF16, tag="E")
                for sb in range(SB):
                    pt = psA.tile([PB, NCH], F32, tag="ps")
                    for kb in range(DB):
                        nc.tensor.matmul(
                            out=pt[:],
                            lhsT=XK[:, kb, sb * PB:(sb + 1) * PB],
                            rhs=GT[:, kb, q0:q0 + NCH],
                            start=(kb == 0),
                            stop=(kb == DB - 1),
                        )
                    nc.scalar.activation(
                        out=E[:, sb, :], in_=pt[:],
                        func=mybir.ActivationFunctionType.Exp,
                        bias=ess[:, sb:sb + 1],
                    )
                # per q-block of 128: denominators r, then U, then epilogue
                for qb in range(NCH // PB):
                    eq = slice(qb * PB, (qb + 1) * PB)
                    pr = psB.tile([PB, 1], F32, tag="pr")
                    for sb in range(SB):
                        nc.tensor.matmul(
                            out=pr[:], lhsT=E[:, sb, eq], rhs=ones[:],
                            start=(sb == 0), stop=(sb == SB - 1),
                        )
                    recip = misc.tile([PB, 1], F32, tag="recip")
                    nc.vector.reciprocal(out=recip[:], in_=pr[:])
                    qrow = q0 + qb * PB
                    last = (qc == QCH - 1) and (qb == NCH // PB - 1)
                    for vc in range(VCH):
                        pu = psB.tile([PB, NCH], F32, tag="pu")
                        for sb in range(SB):
                            nc.tensor.matmul(
                                out=pu[:],
                                lhsT=E[:, sb, eq],
                                rhs=V[:, sb, vc * NCH:(vc + 1) * NCH],
                                start=(sb == 0),
                                stop=(sb == SB - 1),
                            )
                        ot = opool.tile([PB, NCH], F32, tag="ot")
                        # the very last block pipelines finer stt->DMA pieces
                        # so the kernel tail shortens
                        npc = 4 if last else 2
                        h = NCH // npc
                        c0 = vc * NCH
                        for p in range(npc):
                            a = p * h
                            nc.vector.scalar_tensor_tensor(
                                out=ot[:, a:a + h],
                                in0=pu[:, a:a + h],
                                scalar=recip[:],
                                in1=bvB[:, c0 + a:c0 + a + h],
                                op0=mybir.AluOpType.mult,
                                op1=mybir.AluOpType.add,
                            )
                            (nc.sync if p % 2 == 0 else nc.scalar).dma_start(
                                out=out[qrow:qrow + PB, c0 + a:c0 + a + h],
                                in_=ot[:, a:a + h],
                            )

    nc.compile()
    return nc


def _prep_host(query, key_, value, Wq_w, Wq_b, Wk_w, Wk_b, Wv_w, Wv_b):
    """Host-side sharding, layout marshalling, and weight-only algebra."""
    bf16 = ml_dtypes.bfloat16
    scale = np.float32(1.0 / np.sqrt(D))

    def prep_x(x):  # [S, D] fp32 -> [SCH, PB, DB, NCH] bf16 (x.T, blocked)
        # xc[sc, p, kb, n] = x[sc*NCH+n, kb*PB+p]
        return np.ascontiguousarray(
            x.reshape(SCH, NCH, DB, PB).transpose(0, 3, 2, 1)
        ).astype(bf16)

    def prep_w(wT):  # [Din, Dout] fp32 -> [DB, PB, DB, PB] bf16 (blocked)
        # wc[db, p, kb, j] = wT[kb*PB+p, db*PB+j]
        return np.ascontiguousarray(
            wT.reshape(DB, PB, DB, PB).transpose(2, 1, 0, 3)
        ).astype(bf16)

    # weight-only precompute: M = (Wq^T Wk) * scale  [din, din]
    M = (Wq_w.T @ Wk_w) * scale
    mc = prep_w(M)
    wv = prep_w(np.ascontiguousarray(Wv_w.T))
    c = (Wk_w.T @ Wq_b) * scale  # [din]; e_s = xk @ c
    bvr = np.ascontiguousarray(Wv_b.reshape(1, D)).astype(np.float32)

    in_maps = []
    for b in range(B):
        e_s = (key_[b] @ c).astype(np.float32)  # [S]
        in_maps.append({
            "xqT": prep_x(query[b]),
            "xkT": prep_x(key_[b]),
            "xvT": prep_x(value[b]),
            "mT": mc, "wvT": wv,
            "esT": np.ascontiguousarray(e_s.reshape(SB, PB).T),
            "bv": bvr,
        })
    return in_maps


def kernel(query, key_, value, Wq_w, Wq_b, Wk_w, Wk_b, Wv_w, Wv_b):
    query = np.asarray(query, np.float32)
    key_ = np.asarray(key_, np.float32)
    value = np.asarray(value, np.float32)
    Wq_w = np.asarray(Wq_w, np.float32)
    Wq_b = np.asarray(Wq_b, np.float32)
    Wk_w = np.asarray(Wk_w, np.float32)
    Wk_b = np.asarray(Wk_b, np.float32)
    Wv_w = np.asarray(Wv_w, np.float32)
    Wv_b = np.asarray(Wv_b, np.float32)

    if _compiled[0] is None:
        _compiled[0] = _build()
    nc = _compiled[0]

    in_maps = _prep_host(query, key_, value, Wq_w, Wq_b, Wk_w, Wk_b, Wv_w, Wv_b)
    last_err = None
    for attempt in range(3):
        try:
            res = run_bass_kernel_spmd(nc, in_maps, list(range(N_CORES)))
            out = np.stack([res.results[i]["out"] for i in range(N_CORES)], axis=0)
            if np.isfinite(out).all():
                return out
            last_err = RuntimeError("non-finite values in device output")
        except Exception as e:  # transient device errors (e.g. NRT exec unit)
            last_err = e
    raise last_err
